# revision 24
# baseline (speedup 1.0000x reference)
"""Trainium2 Bass kernel for: out = A @ dequant_int4(weight, weight_scale) + bias.

Problem shapes (fp32 A, packed-int4 weight):
    A            [8192, 4096] f32
    weight       [2048, 11008] u8   (two int4 nibbles per byte along K;
                                     row 2i = low nibble, row 2i+1 = high nibble)
    weight_scale [128, 11008] f32   (per-group scale, group_size=32 along K)
    bias         [11008] f32
    out          [8192, 11008] f32
    out = A @ ((nibbles - 8) * scale) + bias

Sharding: tensor-parallel along out_features N across 8 NeuronCores.
Each core gets the full A, a 1376-wide column slice of weight/scale/bias and
computes its [8192, 1376] output slice; the host concatenates slices.

Layout strategy (chosen at shard time on the host, like the column slicing):
  - A ships as blocked bf16 A^T tiles ATB[mc, p, kb, m] = A[128*mc + m, k]
    with k = 256*(kb//2) + 2p + (kb&1) -- i.e. k-block 2b holds even k from
    packed row 128b+p (low nibble), k-block 2b+1 the odd k (high nibble).
    Each m-chunk's lhsT tiles land in SBUF with ONE contiguous 1 MiB DMA and
    the PE never runs a transpose.
  - nibbles are extracted baseline-style (and/shift stay u8->u8 because
    walrus's TensorScalarPtr bitVec ops cannot cast; the subtract-8 is an
    arith op and casts u8->bf16 on output)
  - weight_scale ships row-replicated (x16) as bf16 so the dequant multiply
    is a 2x-mode bf16 tensor_tensor.
  - bias ships as a bf16 row prefixed by 128 ones: bias is added by a K=1
    matmul (ones^T @ bias_row) that OPENS each PSUM accumulation group, so
    the PSUM->SBUF eviction is a plain copy that runs on the scalar engine.

Per-core device program:
  - one-shot dequant of the weight slice into resident SBUF wsb
    [128, 32, 1376] bf16 (k on partitions), n-chunk-major so the PE can
    start while later n-chunks still dequantize;
  - per 128-row chunk of A: one DMA for the lhsT tiles, then per 512-wide
    n-chunk one PSUM group: bias matmul + 32 accumulating matmuls;
  - scalar-engine copy PSUM->SBUF, DMA out.
"""

import numpy as np
import ml_dtypes

import concourse.bacc as bacc
import concourse.tile as tile
from concourse import mybir
from concourse.bass_utils import run_bass_kernel_spmd

M, K, N = 8192, 4096, 11008
NCORES = 8
NS = N // NCORES  # 1376 out-features per core
K2 = K // 2       # 2048 packed rows
P = 128
NB2 = K2 // P     # 16 packed k-blocks
NKB = K // P      # 32 unpacked k-blocks
MCH = M // P      # 64 m-chunks

BF16 = ml_dtypes.bfloat16


def _n_chunks(ns, step=512):
    out = []
    n0 = 0
    while n0 < ns:
        out.append((n0, min(step, ns - n0)))
        n0 += step
    return out


def build_nc(m=M, ns=NS, reps=1, debug=False, no_dma=False, bias_mm_mcs=8,
             no_adma=False, no_evict=False, psum_bufs=8, act_sub=True):
    """Build the per-core Bass program (identical on all cores).

    no_dma: timing probe -- no_adma + no_evict combined.
    no_adma: timing probe -- load A tiles once, reuse for every m-chunk.
    no_evict: timing probe -- skip PSUM evictions + output DMAs.
    bias_mm_mcs: m-chunks [0, bias_mm_mcs) add bias via the K=1 matmul and
        evict with a scalar-engine copy (DVE is still busy dequantizing);
        later m-chunks skip the bias matmul and add bias during a DVE
        tensor_tensor eviction instead, saving PE stream time.
    """
    if no_dma:
        no_adma = no_evict = True
    mch = m // P
    n_chunks = _n_chunks(ns)

    nc = bacc.Bacc(None, target_bir_lowering=False, debug=debug)
    ATB = nc.dram_tensor("atb", [mch, P, NKB, P], mybir.dt.bfloat16, kind="ExternalInput")
    WQB = nc.dram_tensor("wqb", [P, NB2, ns], mybir.dt.uint8, kind="ExternalInput")
    SRB = nc.dram_tensor("srb", [P, NB2, ns], mybir.dt.bfloat16, kind="ExternalInput")
    BIASW = nc.dram_tensor("biasw", [1, P + ns], mybir.dt.bfloat16, kind="ExternalInput")
    BIASR = nc.dram_tensor("biasr", [P, ns], mybir.dt.bfloat16, kind="ExternalInput")
    OUT = nc.dram_tensor("out", [m, ns], mybir.dt.float32, kind="ExternalOutput")

    with tile.TileContext(nc) as tc:
        with (
            tc.tile_pool(name="singles", bufs=1) as singles,
            tc.tile_pool(name="wpool", bufs=1) as wpool,
            tc.tile_pool(name="dq", bufs=3) as dq,
            tc.tile_pool(name="apool", bufs=3) as apool,
            tc.tile_pool(name="opool", bufs=4) as opool,
            tc.tile_pool(name="psum_o", bufs=psum_bufs, space="PSUM") as psum_o,
        ):
            def body():
                # ones (for the bias matmul) + bf16 bias row
                biasw = singles.tile([1, P + ns], mybir.dt.bfloat16, tag="biasw")
                nc.sync.dma_start(out=biasw, in_=BIASW[:, :])
                # partition-replicated bias for the DVE eviction path
                bias_r = singles.tile([P, ns], mybir.dt.bfloat16, tag="bias_r")
                nc.sync.dma_start(out=bias_r, in_=BIASR[:, :])

                pk = singles.tile([P, NB2, ns], mybir.dt.uint8, tag="pk")
                srep = singles.tile([P, NB2, ns], mybir.dt.bfloat16, tag="srep")
                wsb = wpool.tile([P, NKB, ns], mybir.dt.bfloat16, tag="wsb")

                # ---- one-shot dequant, n-chunk-major (DVE only) ----
                for (n0, nch) in n_chunks:
                    nsl = slice(n0, n0 + nch)
                    # per-chunk input DMAs so chunk 0 can start dequantizing
                    # before the rest of the weight slice has landed
                    nc.sync.dma_start(out=pk[:, :, nsl], in_=WQB[:, :, nsl])
                    nc.sync.dma_start(out=srep[:, :, nsl], in_=SRB[:, :, nsl])
                    for b in range(NB2):
                        lq = dq.tile([P, 512], mybir.dt.uint8, tag="lq")
                        hq = dq.tile([P, 512], mybir.dt.uint8, tag="hq")
                        lo = dq.tile([P, 512], mybir.dt.bfloat16, tag="lo")
                        hi = dq.tile([P, 512], mybir.dt.bfloat16, tag="hi")
                        nc.vector.tensor_scalar(
                            out=lq[:, :nch], in0=pk[:, b, nsl], scalar1=15, scalar2=None,
                            op0=mybir.AluOpType.bitwise_and)
                        nc.vector.tensor_scalar(
                            out=hq[:, :nch], in0=pk[:, b, nsl], scalar1=4, scalar2=None,
                            op0=mybir.AluOpType.logical_shift_right)
                        if act_sub:
                            # subtract-8 (+ cast to bf16) runs on the
                            # otherwise idle scalar engine, halving the DVE
                            # dequant path
                            nc.scalar.activation(
                                out=lo[:, :nch], in_=lq[:, :nch],
                                func=mybir.ActivationFunctionType.Copy, bias=-8.0)
                            nc.scalar.activation(
                                out=hi[:, :nch], in_=hq[:, :nch],
                                func=mybir.ActivationFunctionType.Copy, bias=-8.0)
                        else:
                            nc.vector.tensor_scalar(
                                out=lo[:, :nch], in0=lq[:, :nch], scalar1=8,
                                scalar2=None, op0=mybir.AluOpType.subtract)
                            nc.vector.tensor_scalar(
                                out=hi[:, :nch], in0=hq[:, :nch], scalar1=8,
                                scalar2=None, op0=mybir.AluOpType.subtract)
                        nc.vector.tensor_tensor(
                            out=wsb[:, 2 * b, nsl], in0=lo[:, :nch], in1=srep[:, b, nsl],
                            op=mybir.AluOpType.mult)
                        nc.vector.tensor_tensor(
                            out=wsb[:, 2 * b + 1, nsl], in0=hi[:, :nch], in1=srep[:, b, nsl],
                            op=mybir.AluOpType.mult)

                # ---- main loop over 128-row chunks of A ----
                # n-chunk-INNER so consecutive matmuls share the stationary
                # operand (lhsT changes once per kb, not once per matmul);
                # all 3 PSUM banks of an m-chunk accumulate simultaneously.
                at0 = None
                for mc in range(mch):
                    use_mm = mc < bias_mm_mcs
                    if no_adma:
                        if at0 is None:
                            at0 = singles.tile([P, NKB, P], mybir.dt.bfloat16, tag="at0")
                            nc.sync.dma_start(out=at0, in_=ATB[0, :, :, :])
                        at = at0
                    else:
                        at = apool.tile([P, NKB, P], mybir.dt.bfloat16, tag="at")
                        nc.sync.dma_start(out=at, in_=ATB[mc, :, :, :])
                    pos = []
                    for _ in n_chunks:
                        po = psum_o.tile([P, 512], mybir.dt.float32, tag="po")
                        pos.append(po)
                    if use_mm:
                        # bias opens each accumulation group (K=1 rank-1
                        # matmul, shared ones lhsT)
                        for j, (n0, nch) in enumerate(n_chunks):
                            nc.tensor.matmul(
                                pos[j][:, :nch], lhsT=biasw[:, 0:P],
                                rhs=biasw[:, P + n0:P + n0 + nch],
                                start=True, stop=False, skip_group_check=True)
                    for kb in range(NKB):
                        for j, (n0, nch) in enumerate(n_chunks):
                            nc.tensor.matmul(
                                pos[j][:, :nch], lhsT=at[:, kb, :],
                                rhs=wsb[:, kb, n0:n0 + nch],
                                start=(kb == 0 and not use_mm),
                                stop=(kb == NKB - 1),
                                skip_group_check=True)
                    if no_evict:
                        continue
                    for j, (n0, nch) in enumerate(n_chunks):
                        o_sb = opool.tile([P, 512], mybir.dt.float32, tag="o_sb")
                        if use_mm:
                            nc.scalar.copy(out=o_sb[:, :nch], in_=pos[j][:, :nch])
                        else:
                            nc.vector.tensor_tensor(
                                out=o_sb[:, :nch], in0=pos[j][:, :nch],
                                in1=bias_r[:, n0:n0 + nch], op=mybir.AluOpType.add)
                        nc.sync.dma_start(
                            out=OUT[mc * P:(mc + 1) * P, n0:n0 + nch],
                            in_=o_sb[:, :nch])

            if reps == 1:
                body()
            else:
                with tc.For_i(0, reps, 1):
                    body()

    nc.finalize()
    return nc


_NC_CACHE = {}


def _get_nc(reps=1):
    if reps not in _NC_CACHE:
        _NC_CACHE[reps] = build_nc(reps=reps)
    return _NC_CACHE[reps]


def shard_inputs(A, weight, weight_scale, bias):
    A = np.asarray(A, dtype=np.float32)
    wq = np.asarray(weight, dtype=np.uint8)
    ws = np.asarray(weight_scale, dtype=np.float32)
    bs = np.asarray(bias, dtype=np.float32)

    # Blocked bf16 A^T tiles: ATB[mc, p, (b, t), m] = A[128 mc + m, 256 b + 2 p + t]
    atb = A.astype(BF16).reshape(MCH, P, NB2, P, 2)          # (mc, m, b, p, t)
    atb = np.ascontiguousarray(atb.transpose(0, 3, 2, 4, 1)) # (mc, p, b, t, m)
    atb = atb.reshape(MCH, P, NKB, P)

    srep = np.repeat(ws, 16, axis=0).astype(BF16)            # [2048, N]

    in_maps = []
    for c in range(NCORES):
        sl = slice(c * NS, (c + 1) * NS)
        # [2048, ns] -> [128, 16, ns] with row 128 b + p on (p, b)
        wqb = np.ascontiguousarray(
            wq[:, sl].reshape(NB2, P, NS).transpose(1, 0, 2))
        srb = np.ascontiguousarray(
            srep[:, sl].reshape(NB2, P, NS).transpose(1, 0, 2))
        biasw = np.concatenate(
            [np.ones(P, dtype=BF16), bs[sl].astype(BF16)]).reshape(1, P + NS)
        biasr = np.ascontiguousarray(
            np.broadcast_to(bs[sl].astype(BF16), (P, NS)))
        in_maps.append({"atb": atb, "wqb": wqb, "srb": srb, "biasw": biasw,
                        "biasr": biasr})
    return in_maps


def run(inputs, trace=False, reps=1, **kw):
    nc = _get_nc(reps)
    in_maps = shard_inputs(**inputs)
    res = run_bass_kernel_spmd(nc, in_maps, core_ids=list(range(NCORES)), trace=trace, **kw)
    out = np.concatenate([res.results[c]["out"] for c in range(NCORES)], axis=1)
    return out, res


def kernel(A, weight, weight_scale, bias):
    out, _ = run(dict(A=A, weight=weight, weight_scale=weight_scale, bias=bias))
    return out


# revision 26
# speedup vs baseline: 1.0182x; 1.0182x over previous
"""Trainium2 Bass kernel for: out = A @ dequant_int4(weight, weight_scale) + bias.

Problem shapes (fp32 A, packed-int4 weight):
    A            [8192, 4096] f32
    weight       [2048, 11008] u8   (two int4 nibbles per byte along K;
                                     row 2i = low nibble, row 2i+1 = high nibble)
    weight_scale [128, 11008] f32   (per-group scale, group_size=32 along K)
    bias         [11008] f32
    out          [8192, 11008] f32
    out = A @ ((nibbles - 8) * scale) + bias

Sharding: tensor-parallel along out_features N across 8 NeuronCores.
Each core gets the full A, a 1376-wide column slice of weight/scale/bias and
computes its [8192, 1376] output slice; the host concatenates slices.

Layout strategy (chosen at shard time on the host, like the column slicing):
  - A ships as blocked bf16 A^T tiles ATB[mc, p, kb, m] = A[128*mc + m, k]
    with k = 256*(kb//2) + 2p + (kb&1) -- i.e. k-block 2b holds even k from
    packed row 128b+p (low nibble), k-block 2b+1 the odd k (high nibble).
    Each m-chunk's lhsT tiles land in SBUF with ONE contiguous 1 MiB DMA and
    the PE never runs a transpose.
  - nibbles are extracted baseline-style (and/shift stay u8->u8 because
    walrus's TensorScalarPtr bitVec ops cannot cast; the subtract-8 is an
    arith op and casts u8->bf16 on output)
  - weight_scale ships row-replicated (x16) as bf16 so the dequant multiply
    is a 2x-mode bf16 tensor_tensor.
  - bias ships as a bf16 row prefixed by 128 ones: bias is added by a K=1
    matmul (ones^T @ bias_row) that OPENS each PSUM accumulation group, so
    the PSUM->SBUF eviction is a plain copy that runs on the scalar engine.

Per-core device program:
  - one-shot dequant of the weight slice into resident SBUF wsb
    [128, 32, 1376] bf16 (k on partitions), n-chunk-major so the PE can
    start while later n-chunks still dequantize;
  - per 128-row chunk of A: one DMA for the lhsT tiles, then per 512-wide
    n-chunk one PSUM group: bias matmul + 32 accumulating matmuls;
  - scalar-engine copy PSUM->SBUF, DMA out.
"""

import numpy as np
import ml_dtypes

import concourse.bacc as bacc
import concourse.tile as tile
from concourse import mybir
from concourse.bass_utils import run_bass_kernel_spmd

M, K, N = 8192, 4096, 11008
NCORES = 8
NS = N // NCORES  # 1376 out-features per core
K2 = K // 2       # 2048 packed rows
P = 128
NB2 = K2 // P     # 16 packed k-blocks
NKB = K // P      # 32 unpacked k-blocks
MCH = M // P      # 64 m-chunks

BF16 = ml_dtypes.bfloat16


def _n_chunks(ns, step=512):
    out = []
    n0 = 0
    while n0 < ns:
        out.append((n0, min(step, ns - n0)))
        n0 += step
    return out


def build_nc(m=M, ns=NS, reps=1, debug=False, no_dma=False, bias_mm_mcs=8,
             no_adma=False, no_evict=False, psum_bufs=8, act_sub=True):
    """Build the per-core Bass program (identical on all cores).

    no_dma: timing probe -- no_adma + no_evict combined.
    no_adma: timing probe -- load A tiles once, reuse for every m-chunk.
    no_evict: timing probe -- skip PSUM evictions + output DMAs.
    bias_mm_mcs: m-chunks [0, bias_mm_mcs) add bias via the K=1 matmul and
        evict with a scalar-engine copy (DVE is still busy dequantizing);
        later m-chunks skip the bias matmul and add bias during a DVE
        tensor_tensor eviction instead, saving PE stream time.
    """
    if no_dma:
        no_adma = no_evict = True
    mch = m // P
    n_chunks = _n_chunks(ns)

    nc = bacc.Bacc(None, target_bir_lowering=False, debug=debug)
    ATB = nc.dram_tensor("atb", [mch, P, NKB, P], mybir.dt.bfloat16, kind="ExternalInput")
    WQB = nc.dram_tensor("wqb", [P, NB2, ns], mybir.dt.uint8, kind="ExternalInput")
    SRB = nc.dram_tensor("srb", [P, NB2, ns], mybir.dt.bfloat16, kind="ExternalInput")
    BIASW = nc.dram_tensor("biasw", [1, P + ns], mybir.dt.bfloat16, kind="ExternalInput")
    BIASR = nc.dram_tensor("biasr", [P, ns], mybir.dt.bfloat16, kind="ExternalInput")
    OUT = nc.dram_tensor("out", [m, ns], mybir.dt.float32, kind="ExternalOutput")

    with tile.TileContext(nc) as tc:
        with (
            tc.tile_pool(name="singles", bufs=1) as singles,
            tc.tile_pool(name="wpool", bufs=1) as wpool,
            tc.tile_pool(name="dq", bufs=3) as dq,
            tc.tile_pool(name="apool", bufs=3) as apool,
            tc.tile_pool(name="opool", bufs=4) as opool,
            tc.tile_pool(name="psum_o", bufs=psum_bufs, space="PSUM") as psum_o,
        ):
            def body():
                # ones (for the bias matmul) + bf16 bias row
                biasw = singles.tile([1, P + ns], mybir.dt.bfloat16, tag="biasw")
                nc.sync.dma_start(out=biasw, in_=BIASW[:, :])
                # partition-replicated bias for the DVE eviction path
                bias_r = singles.tile([P, ns], mybir.dt.bfloat16, tag="bias_r")
                nc.sync.dma_start(out=bias_r, in_=BIASR[:, :])

                pk = singles.tile([P, NB2, ns], mybir.dt.uint8, tag="pk")
                srep = singles.tile([P, NB2, ns], mybir.dt.bfloat16, tag="srep")
                wsb = wpool.tile([P, NKB, ns], mybir.dt.bfloat16, tag="wsb")

                # ---- one-shot dequant, n-chunk-major (DVE only) ----
                at_pre = []
                for ci, (n0, nch) in enumerate(n_chunks):
                    nsl = slice(n0, n0 + nch)
                    # per-chunk input DMAs so chunk 0 can start dequantizing
                    # before the rest of the weight slice has landed
                    nc.sync.dma_start(out=pk[:, :, nsl], in_=WQB[:, :, nsl])
                    nc.sync.dma_start(out=srep[:, :, nsl], in_=SRB[:, :, nsl])
                    if ci == 0 and not no_adma:
                        # prefetch the first A tiles ahead of the remaining
                        # weight/scale DMAs (the rings are FIFO, and the
                        # first matmul group needs at(mc0) as much as wsb)
                        for pmc in range(min(2, mch)):
                            at = apool.tile([P, NKB, P], mybir.dt.bfloat16, tag="at")
                            nc.sync.dma_start(out=at, in_=ATB[pmc, :, :, :])
                            at_pre.append(at)
                    for b in range(NB2):
                        lq = dq.tile([P, 512], mybir.dt.uint8, tag="lq")
                        hq = dq.tile([P, 512], mybir.dt.uint8, tag="hq")
                        lo = dq.tile([P, 512], mybir.dt.bfloat16, tag="lo")
                        hi = dq.tile([P, 512], mybir.dt.bfloat16, tag="hi")
                        nc.vector.tensor_scalar(
                            out=lq[:, :nch], in0=pk[:, b, nsl], scalar1=15, scalar2=None,
                            op0=mybir.AluOpType.bitwise_and)
                        nc.vector.tensor_scalar(
                            out=hq[:, :nch], in0=pk[:, b, nsl], scalar1=4, scalar2=None,
                            op0=mybir.AluOpType.logical_shift_right)
                        if act_sub:
                            # subtract-8 (+ cast to bf16) runs on the
                            # otherwise idle scalar engine, halving the DVE
                            # dequant path
                            nc.scalar.activation(
                                out=lo[:, :nch], in_=lq[:, :nch],
                                func=mybir.ActivationFunctionType.Copy, bias=-8.0)
                            nc.scalar.activation(
                                out=hi[:, :nch], in_=hq[:, :nch],
                                func=mybir.ActivationFunctionType.Copy, bias=-8.0)
                        else:
                            nc.vector.tensor_scalar(
                                out=lo[:, :nch], in0=lq[:, :nch], scalar1=8,
                                scalar2=None, op0=mybir.AluOpType.subtract)
                            nc.vector.tensor_scalar(
                                out=hi[:, :nch], in0=hq[:, :nch], scalar1=8,
                                scalar2=None, op0=mybir.AluOpType.subtract)
                        nc.vector.tensor_tensor(
                            out=wsb[:, 2 * b, nsl], in0=lo[:, :nch], in1=srep[:, b, nsl],
                            op=mybir.AluOpType.mult)
                        nc.vector.tensor_tensor(
                            out=wsb[:, 2 * b + 1, nsl], in0=hi[:, :nch], in1=srep[:, b, nsl],
                            op=mybir.AluOpType.mult)

                # ---- main loop over 128-row chunks of A ----
                # n-chunk-INNER so consecutive matmuls share the stationary
                # operand (lhsT changes once per kb, not once per matmul);
                # all 3 PSUM banks of an m-chunk accumulate simultaneously.
                at0 = None
                for mc in range(mch):
                    use_mm = mc < bias_mm_mcs
                    if no_adma:
                        if at0 is None:
                            at0 = singles.tile([P, NKB, P], mybir.dt.bfloat16, tag="at0")
                            nc.sync.dma_start(out=at0, in_=ATB[0, :, :, :])
                        at = at0
                    elif mc < len(at_pre):
                        at = at_pre[mc]
                    else:
                        at = apool.tile([P, NKB, P], mybir.dt.bfloat16, tag="at")
                        nc.sync.dma_start(out=at, in_=ATB[mc, :, :, :])
                    pos = []
                    for _ in n_chunks:
                        po = psum_o.tile([P, 512], mybir.dt.float32, tag="po")
                        pos.append(po)
                    if use_mm:
                        # bias opens each accumulation group (K=1 rank-1
                        # matmul, shared ones lhsT)
                        for j, (n0, nch) in enumerate(n_chunks):
                            nc.tensor.matmul(
                                pos[j][:, :nch], lhsT=biasw[:, 0:P],
                                rhs=biasw[:, P + n0:P + n0 + nch],
                                start=True, stop=False, skip_group_check=True)
                    for kb in range(NKB):
                        for j, (n0, nch) in enumerate(n_chunks):
                            nc.tensor.matmul(
                                pos[j][:, :nch], lhsT=at[:, kb, :],
                                rhs=wsb[:, kb, n0:n0 + nch],
                                start=(kb == 0 and not use_mm),
                                stop=(kb == NKB - 1),
                                skip_group_check=True)
                    if no_evict:
                        continue
                    for j, (n0, nch) in enumerate(n_chunks):
                        o_sb = opool.tile([P, 512], mybir.dt.float32, tag="o_sb")
                        if use_mm:
                            nc.scalar.copy(out=o_sb[:, :nch], in_=pos[j][:, :nch])
                        else:
                            nc.vector.tensor_tensor(
                                out=o_sb[:, :nch], in0=pos[j][:, :nch],
                                in1=bias_r[:, n0:n0 + nch], op=mybir.AluOpType.add)
                        nc.sync.dma_start(
                            out=OUT[mc * P:(mc + 1) * P, n0:n0 + nch],
                            in_=o_sb[:, :nch])

            if reps == 1:
                body()
            else:
                with tc.For_i(0, reps, 1):
                    body()

    nc.finalize()
    return nc


_NC_CACHE = {}


def _get_nc(reps=1):
    if reps not in _NC_CACHE:
        _NC_CACHE[reps] = build_nc(reps=reps)
    return _NC_CACHE[reps]


def shard_inputs(A, weight, weight_scale, bias):
    A = np.asarray(A, dtype=np.float32)
    wq = np.asarray(weight, dtype=np.uint8)
    ws = np.asarray(weight_scale, dtype=np.float32)
    bs = np.asarray(bias, dtype=np.float32)

    # Blocked bf16 A^T tiles: ATB[mc, p, (b, t), m] = A[128 mc + m, 256 b + 2 p + t]
    atb = A.astype(BF16).reshape(MCH, P, NB2, P, 2)          # (mc, m, b, p, t)
    atb = np.ascontiguousarray(atb.transpose(0, 3, 2, 4, 1)) # (mc, p, b, t, m)
    atb = atb.reshape(MCH, P, NKB, P)

    srep = np.repeat(ws, 16, axis=0).astype(BF16)            # [2048, N]

    in_maps = []
    for c in range(NCORES):
        sl = slice(c * NS, (c + 1) * NS)
        # [2048, ns] -> [128, 16, ns] with row 128 b + p on (p, b)
        wqb = np.ascontiguousarray(
            wq[:, sl].reshape(NB2, P, NS).transpose(1, 0, 2))
        srb = np.ascontiguousarray(
            srep[:, sl].reshape(NB2, P, NS).transpose(1, 0, 2))
        biasw = np.concatenate(
            [np.ones(P, dtype=BF16), bs[sl].astype(BF16)]).reshape(1, P + NS)
        biasr = np.ascontiguousarray(
            np.broadcast_to(bs[sl].astype(BF16), (P, NS)))
        in_maps.append({"atb": atb, "wqb": wqb, "srb": srb, "biasw": biasw,
                        "biasr": biasr})
    return in_maps


def run(inputs, trace=False, reps=1, **kw):
    nc = _get_nc(reps)
    in_maps = shard_inputs(**inputs)
    res = run_bass_kernel_spmd(nc, in_maps, core_ids=list(range(NCORES)), trace=trace, **kw)
    out = np.concatenate([res.results[c]["out"] for c in range(NCORES)], axis=1)
    return out, res


def kernel(A, weight, weight_scale, bias):
    out, _ = run(dict(A=A, weight=weight, weight_scale=weight_scale, bias=bias))
    return out


# revision 28
# speedup vs baseline: 1.0224x; 1.0041x over previous
"""Trainium2 Bass kernel for: out = A @ dequant_int4(weight, weight_scale) + bias.

Problem shapes (fp32 A, packed-int4 weight):
    A            [8192, 4096] f32
    weight       [2048, 11008] u8   (two int4 nibbles per byte along K;
                                     row 2i = low nibble, row 2i+1 = high nibble)
    weight_scale [128, 11008] f32   (per-group scale, group_size=32 along K)
    bias         [11008] f32
    out          [8192, 11008] f32
    out = A @ ((nibbles - 8) * scale) + bias

Sharding: tensor-parallel along out_features N across 8 NeuronCores.
Each core gets the full A, a 1376-wide column slice of weight/scale/bias and
computes its [8192, 1376] output slice; the host concatenates slices.

Layout strategy (chosen at shard time on the host, like the column slicing):
  - A ships as blocked bf16 A^T tiles ATB[mc, p, kb, m] = A[128*mc + m, k]
    with k = 256*(kb//2) + 2p + (kb&1) -- i.e. k-block 2b holds even k from
    packed row 128b+p (low nibble), k-block 2b+1 the odd k (high nibble).
    Each m-chunk's lhsT tiles land in SBUF with ONE contiguous 1 MiB DMA and
    the PE never runs a transpose.
  - nibbles are extracted baseline-style (and/shift stay u8->u8 because
    walrus's TensorScalarPtr bitVec ops cannot cast; the subtract-8 is an
    arith op and casts u8->bf16 on output)
  - weight_scale ships row-replicated (x16) as bf16 so the dequant multiply
    is a 2x-mode bf16 tensor_tensor.
  - bias ships as a bf16 row prefixed by 128 ones: bias is added by a K=1
    matmul (ones^T @ bias_row) that OPENS each PSUM accumulation group, so
    the PSUM->SBUF eviction is a plain copy that runs on the scalar engine.

Per-core device program:
  - one-shot dequant of the weight slice into resident SBUF wsb
    [128, 32, 1376] bf16 (k on partitions), n-chunk-major so the PE can
    start while later n-chunks still dequantize;
  - per 128-row chunk of A: one DMA for the lhsT tiles, then per 512-wide
    n-chunk one PSUM group: bias matmul + 32 accumulating matmuls;
  - scalar-engine copy PSUM->SBUF, DMA out.
"""

import numpy as np
import ml_dtypes

import concourse.bacc as bacc
import concourse.tile as tile
from concourse import mybir
from concourse.bass_utils import run_bass_kernel_spmd

M, K, N = 8192, 4096, 11008
NCORES = 8
NS = N // NCORES  # 1376 out-features per core
K2 = K // 2       # 2048 packed rows
P = 128
NB2 = K2 // P     # 16 packed k-blocks
NKB = K // P      # 32 unpacked k-blocks
MCH = M // P      # 64 m-chunks

BF16 = ml_dtypes.bfloat16


def _n_chunks(ns, step=512):
    out = []
    n0 = 0
    while n0 < ns:
        out.append((n0, min(step, ns - n0)))
        n0 += step
    # narrowest chunk first: it dequantizes ~30% faster, so the PE's first
    # accumulation group becomes ready sooner
    out.sort(key=lambda c: c[1])
    return out


def build_nc(m=M, ns=NS, reps=1, debug=False, no_dma=False, bias_mm_mcs=8,
             no_adma=False, no_evict=False, psum_bufs=8, act_sub=True):
    """Build the per-core Bass program (identical on all cores).

    no_dma: timing probe -- no_adma + no_evict combined.
    no_adma: timing probe -- load A tiles once, reuse for every m-chunk.
    no_evict: timing probe -- skip PSUM evictions + output DMAs.
    bias_mm_mcs: m-chunks [0, bias_mm_mcs) add bias via the K=1 matmul and
        evict with a scalar-engine copy (DVE is still busy dequantizing);
        later m-chunks skip the bias matmul and add bias during a DVE
        tensor_tensor eviction instead, saving PE stream time.
    """
    if no_dma:
        no_adma = no_evict = True
    mch = m // P
    n_chunks = _n_chunks(ns)

    nc = bacc.Bacc(None, target_bir_lowering=False, debug=debug)
    ATB = nc.dram_tensor("atb", [mch, P, NKB, P], mybir.dt.bfloat16, kind="ExternalInput")
    WQB = nc.dram_tensor("wqb", [P, NB2, ns], mybir.dt.uint8, kind="ExternalInput")
    SRB = nc.dram_tensor("srb", [P, NB2, ns], mybir.dt.bfloat16, kind="ExternalInput")
    BIASW = nc.dram_tensor("biasw", [1, P + ns], mybir.dt.bfloat16, kind="ExternalInput")
    BIASR = nc.dram_tensor("biasr", [P, ns], mybir.dt.bfloat16, kind="ExternalInput")
    OUT = nc.dram_tensor("out", [m, ns], mybir.dt.float32, kind="ExternalOutput")

    with tile.TileContext(nc) as tc:
        with (
            tc.tile_pool(name="singles", bufs=1) as singles,
            tc.tile_pool(name="wpool", bufs=1) as wpool,
            tc.tile_pool(name="dq", bufs=3) as dq,
            tc.tile_pool(name="apool", bufs=3) as apool,
            tc.tile_pool(name="opool", bufs=4) as opool,
            tc.tile_pool(name="psum_o", bufs=psum_bufs, space="PSUM") as psum_o,
        ):
            def body():
                # ones (for the bias matmul) + bf16 bias row
                biasw = singles.tile([1, P + ns], mybir.dt.bfloat16, tag="biasw")
                nc.sync.dma_start(out=biasw, in_=BIASW[:, :])
                # partition-replicated bias for the DVE eviction path
                bias_r = singles.tile([P, ns], mybir.dt.bfloat16, tag="bias_r")
                nc.sync.dma_start(out=bias_r, in_=BIASR[:, :])

                pk = singles.tile([P, NB2, ns], mybir.dt.uint8, tag="pk")
                srep = singles.tile([P, NB2, ns], mybir.dt.bfloat16, tag="srep")
                wsb = wpool.tile([P, NKB, ns], mybir.dt.bfloat16, tag="wsb")

                # ---- one-shot dequant, n-chunk-major (DVE only) ----
                at_pre = []
                for ci, (n0, nch) in enumerate(n_chunks):
                    nsl = slice(n0, n0 + nch)
                    # per-chunk input DMAs so chunk 0 can start dequantizing
                    # before the rest of the weight slice has landed; the
                    # first chunk's DMAs land in b-halves so the very first
                    # dequant op waits on 0.5 MB, not 3 MB
                    if ci == 0:
                        h = NB2 // 2
                        nc.sync.dma_start(out=pk[:, 0:h, nsl], in_=WQB[:, 0:h, nsl])
                        nc.sync.dma_start(out=srep[:, 0:h, nsl], in_=SRB[:, 0:h, nsl])
                        nc.sync.dma_start(out=pk[:, h:, nsl], in_=WQB[:, h:, nsl])
                        nc.sync.dma_start(out=srep[:, h:, nsl], in_=SRB[:, h:, nsl])
                    else:
                        nc.sync.dma_start(out=pk[:, :, nsl], in_=WQB[:, :, nsl])
                        nc.sync.dma_start(out=srep[:, :, nsl], in_=SRB[:, :, nsl])
                    if ci == 0 and not no_adma:
                        # prefetch the first A tiles ahead of the remaining
                        # weight/scale DMAs (the rings are FIFO, and the
                        # first matmul group needs at(mc0) as much as wsb)
                        for pmc in range(min(2, mch)):
                            at = apool.tile([P, NKB, P], mybir.dt.bfloat16, tag="at")
                            nc.sync.dma_start(out=at, in_=ATB[pmc, :, :, :])
                            at_pre.append(at)
                    for b in range(NB2):
                        lq = dq.tile([P, 512], mybir.dt.uint8, tag="lq")
                        hq = dq.tile([P, 512], mybir.dt.uint8, tag="hq")
                        lo = dq.tile([P, 512], mybir.dt.bfloat16, tag="lo")
                        hi = dq.tile([P, 512], mybir.dt.bfloat16, tag="hi")
                        nc.vector.tensor_scalar(
                            out=lq[:, :nch], in0=pk[:, b, nsl], scalar1=15, scalar2=None,
                            op0=mybir.AluOpType.bitwise_and)
                        nc.vector.tensor_scalar(
                            out=hq[:, :nch], in0=pk[:, b, nsl], scalar1=4, scalar2=None,
                            op0=mybir.AluOpType.logical_shift_right)
                        if act_sub:
                            # subtract-8 (+ cast to bf16) runs on the
                            # otherwise idle scalar engine, halving the DVE
                            # dequant path
                            nc.scalar.activation(
                                out=lo[:, :nch], in_=lq[:, :nch],
                                func=mybir.ActivationFunctionType.Copy, bias=-8.0)
                            nc.scalar.activation(
                                out=hi[:, :nch], in_=hq[:, :nch],
                                func=mybir.ActivationFunctionType.Copy, bias=-8.0)
                        else:
                            nc.vector.tensor_scalar(
                                out=lo[:, :nch], in0=lq[:, :nch], scalar1=8,
                                scalar2=None, op0=mybir.AluOpType.subtract)
                            nc.vector.tensor_scalar(
                                out=hi[:, :nch], in0=hq[:, :nch], scalar1=8,
                                scalar2=None, op0=mybir.AluOpType.subtract)
                        nc.vector.tensor_tensor(
                            out=wsb[:, 2 * b, nsl], in0=lo[:, :nch], in1=srep[:, b, nsl],
                            op=mybir.AluOpType.mult)
                        nc.vector.tensor_tensor(
                            out=wsb[:, 2 * b + 1, nsl], in0=hi[:, :nch], in1=srep[:, b, nsl],
                            op=mybir.AluOpType.mult)

                # ---- main loop over 128-row chunks of A ----
                # n-chunk-INNER so consecutive matmuls share the stationary
                # operand (lhsT changes once per kb, not once per matmul);
                # all 3 PSUM banks of an m-chunk accumulate simultaneously.
                at0 = None
                for mc in range(mch):
                    use_mm = mc < bias_mm_mcs
                    if no_adma:
                        if at0 is None:
                            at0 = singles.tile([P, NKB, P], mybir.dt.bfloat16, tag="at0")
                            nc.sync.dma_start(out=at0, in_=ATB[0, :, :, :])
                        at = at0
                    elif mc < len(at_pre):
                        at = at_pre[mc]
                    else:
                        at = apool.tile([P, NKB, P], mybir.dt.bfloat16, tag="at")
                        nc.sync.dma_start(out=at, in_=ATB[mc, :, :, :])
                    pos = []
                    for _ in n_chunks:
                        po = psum_o.tile([P, 512], mybir.dt.float32, tag="po")
                        pos.append(po)
                    if use_mm:
                        # bias opens each accumulation group (K=1 rank-1
                        # matmul, shared ones lhsT)
                        for j, (n0, nch) in enumerate(n_chunks):
                            nc.tensor.matmul(
                                pos[j][:, :nch], lhsT=biasw[:, 0:P],
                                rhs=biasw[:, P + n0:P + n0 + nch],
                                start=True, stop=False, skip_group_check=True)
                    for kb in range(NKB):
                        for j, (n0, nch) in enumerate(n_chunks):
                            nc.tensor.matmul(
                                pos[j][:, :nch], lhsT=at[:, kb, :],
                                rhs=wsb[:, kb, n0:n0 + nch],
                                start=(kb == 0 and not use_mm),
                                stop=(kb == NKB - 1),
                                skip_group_check=True)
                    if no_evict:
                        continue
                    for j, (n0, nch) in enumerate(n_chunks):
                        o_sb = opool.tile([P, 512], mybir.dt.float32, tag="o_sb")
                        if use_mm:
                            nc.scalar.copy(out=o_sb[:, :nch], in_=pos[j][:, :nch])
                        else:
                            nc.vector.tensor_tensor(
                                out=o_sb[:, :nch], in0=pos[j][:, :nch],
                                in1=bias_r[:, n0:n0 + nch], op=mybir.AluOpType.add)
                        nc.sync.dma_start(
                            out=OUT[mc * P:(mc + 1) * P, n0:n0 + nch],
                            in_=o_sb[:, :nch])

            if reps == 1:
                body()
            else:
                with tc.For_i(0, reps, 1):
                    body()

    nc.finalize()
    return nc


_NC_CACHE = {}


def _get_nc(reps=1):
    if reps not in _NC_CACHE:
        _NC_CACHE[reps] = build_nc(reps=reps)
    return _NC_CACHE[reps]


def shard_inputs(A, weight, weight_scale, bias):
    A = np.asarray(A, dtype=np.float32)
    wq = np.asarray(weight, dtype=np.uint8)
    ws = np.asarray(weight_scale, dtype=np.float32)
    bs = np.asarray(bias, dtype=np.float32)

    # Blocked bf16 A^T tiles: ATB[mc, p, (b, t), m] = A[128 mc + m, 256 b + 2 p + t]
    atb = A.astype(BF16).reshape(MCH, P, NB2, P, 2)          # (mc, m, b, p, t)
    atb = np.ascontiguousarray(atb.transpose(0, 3, 2, 4, 1)) # (mc, p, b, t, m)
    atb = atb.reshape(MCH, P, NKB, P)

    srep = np.repeat(ws, 16, axis=0).astype(BF16)            # [2048, N]

    in_maps = []
    for c in range(NCORES):
        sl = slice(c * NS, (c + 1) * NS)
        # [2048, ns] -> [128, 16, ns] with row 128 b + p on (p, b)
        wqb = np.ascontiguousarray(
            wq[:, sl].reshape(NB2, P, NS).transpose(1, 0, 2))
        srb = np.ascontiguousarray(
            srep[:, sl].reshape(NB2, P, NS).transpose(1, 0, 2))
        biasw = np.concatenate(
            [np.ones(P, dtype=BF16), bs[sl].astype(BF16)]).reshape(1, P + NS)
        biasr = np.ascontiguousarray(
            np.broadcast_to(bs[sl].astype(BF16), (P, NS)))
        in_maps.append({"atb": atb, "wqb": wqb, "srb": srb, "biasw": biasw,
                        "biasr": biasr})
    return in_maps


def run(inputs, trace=False, reps=1, **kw):
    nc = _get_nc(reps)
    in_maps = shard_inputs(**inputs)
    res = run_bass_kernel_spmd(nc, in_maps, core_ids=list(range(NCORES)), trace=trace, **kw)
    out = np.concatenate([res.results[c]["out"] for c in range(NCORES)], axis=1)
    return out, res


def kernel(A, weight, weight_scale, bias):
    out, _ = run(dict(A=A, weight=weight, weight_scale=weight_scale, bias=bias))
    return out
